# revision 2
# baseline (speedup 1.0000x reference)
"""Batch triplet loss on 8 TRN2 NeuronCores — v11: fold + triangle + TT-tree.

v10 -> v11:
- Own-panel triangle: slab 0 (own cols [0,512)) is computed only for
  its 0-3; the dropped (its 4-7, s0) pairs are recovered from the
  transpose side by giving slab 1 facc treatment (col-max over its 0-3),
  whose segment maps to own rows [512,1024).  -16 matmuls.
- m1 row maxes via a per-it accumulator: d2t tiles TT-max (f16 2x mode,
  ~448ns) into rowacc, one tensor_reduce per it (~615ns) instead of one
  reduce per tile.  m1cols/msb two-stage reduce eliminated.
- facc/rowacc initialized by tensor_copy of the first tile instead of
  memset + TT.
- m2 segment copies moved from ScalarE to gpsimd SBUF->SBUF DMA.
"""

import os
from contextlib import ExitStack

import ml_dtypes
import numpy as np

import concourse.bass as bass
import concourse.tile as tile
from concourse import bacc, bass_isa, bass_utils, mybir

N = 8192
D = 1024
NCORES = 8
OWN = N // NCORES       # 1024
KT = D // 128           # 8
JW = 512
NPAN = 5
MOVW = NPAN * OWN       # 5120
NSLAB = MOVW // JW      # 10
IT = OWN // 128         # 8
EPS = 1e-6
MARGIN = 0.5

ALPHA = 16.0
C0 = 512.0
PAD = 8.0

F8 = mybir.dt.float8e4
F16 = mybir.dt.float16
F32 = mybir.dt.float32

_NC = None

D2S = set(range(1, 10))          # slabs with col-max (facc) treatment
NSEG = 9


def _slab_its(s):
    if s == 0:
        return range(0, IT // 2)     # triangle: own cols A only vs rows A
    if s == 8:
        return range(0, IT // 2)
    if s == 9:
        return range(IT // 2, IT)
    return range(IT)


def _chunks(it):
    if it < IT // 2:
        return [[0, 1, 2, 3, 4], [5, 6, 7, 8]]
    return [[1, 2, 3, 4], [5, 6, 7, 9]]


def _facc_its(s):
    # which its contribute to facc[s] (col-max accumulation)
    if s == 1:
        return range(0, IT // 2)     # recovers dropped (its4-7, s0) pairs
    return _slab_its(s)


def _build_nc():
    REPEAT = int(os.environ.get("KBENCH_REPEAT", "1"))
    HWLOOP = int(os.environ.get("KBENCH_HWLOOP", "0"))  # hw-loop pair count
    nc = bacc.Bacc("TRN2", target_bir_lowering=False, debug=False)
    mov = nc.dram_tensor("mov", [128, KT * MOVW], F8, kind="ExternalInput").ap()
    movl = nc.dram_tensor("movl", [128, KT * OWN], F8, kind="ExternalInput").ap()
    sqbd = nc.dram_tensor("sqbd", [128, IT], F32, kind="ExternalInput").ap()
    out_m1 = nc.dram_tensor("out_m1", [128, IT], F32, kind="ExternalOutput").ap()
    out_m2 = nc.dram_tensor("out_m2", [1, NSEG * JW], F16, kind="ExternalOutput").ap()

    mov_v = mov.rearrange("p (k w) -> p k w", k=KT)    # [128, KT, MOVW]

    with ExitStack() as ctx:
        tc = ctx.enter_context(tile.TileContext(nc))
        big = ctx.enter_context(tc.tile_pool(name="big", bufs=1))
        d2p = ctx.enter_context(tc.tile_pool(name="d2p", bufs=10))
        rowp = ctx.enter_context(tc.tile_pool(name="rowp", bufs=3))
        facp = ctx.enter_context(tc.tile_pool(name="facp", bufs=18))
        parp = ctx.enter_context(tc.tile_pool(name="parp", bufs=4))
        resp = ctx.enter_context(tc.tile_pool(name="resp", bufs=1))
        ps_mm = ctx.enter_context(tc.tile_pool(name="ps_mm", bufs=8, space="PSUM"))

        movs2 = [
            big.tile([128, KT * MOVW], F8, tag=f"mv{h}", name=f"movs_all{h}")
            for h in range(2)
        ]
        movl2 = [
            big.tile([128, KT * OWN], F8, tag=f"ml{h}", name=f"movl{h}")
            for h in range(2)
        ]
        sqb2 = [
            resp.tile([128, IT], F32, tag=f"sqb{h}", name=f"sqb{h}") for h in range(2)
        ]
        msb = resp.tile([128, IT], F32, tag="msb", name="msb")
        m2sb = resp.tile([1, NSEG * JW], F16, tag="m2sb", name="m2sb")

        def emit_rep(rep_i):
            mv = movs2[rep_i % 2][:].rearrange("p (k w) -> p k w", k=KT)
            ml = movl2[rep_i % 2][:].rearrange("p (k w) -> p k w", k=KT)
            sqb = sqb2[rep_i % 2]
            nc.sync.dma_start(sqb[:], sqbd[:])
            nc.sync.dma_start(movl2[rep_i % 2][:], movl[:])
            for n in range(NPAN):
                nc.sync.dma_start(
                    mv[:, :, n * OWN : (n + 1) * OWN],
                    mov_v[:, :, n * OWN : (n + 1) * OWN],
                )

            facc = {}

            def emit_ar(s):
                par = parp.tile([128, JW], F16, tag="par", name=f"par{rep_i}_{s}")
                nc.gpsimd.partition_all_reduce(
                    par[:], facc[s][:], channels=128, reduce_op=bass_isa.ReduceOp.max
                )
                nc.gpsimd.dma_start(
                    m2sb[:, (s - 1) * JW : s * JW], par[0:1, :]
                )

            for it in range(IT):
                rowacc = None
                for chunk in _chunks(it):
                    active = [s for s in chunk if it in _slab_its(s)]
                    psds = {}
                    for s in active:
                        psds[s] = ps_mm.tile(
                            [128, JW], F32, tag="psd", name=f"psd{rep_i}_{it}_{s}"
                        )
                    for t in range(KT // 2):
                        for s in active:
                            nc.tensor.matmul(
                                psds[s][:],
                                ml[:, 2 * t : 2 * t + 2, it * 128 : (it + 1) * 128],
                                mv[:, 2 * t : 2 * t + 2, s * JW : (s + 1) * JW],
                                start=(t == 0),
                                stop=(t == KT // 2 - 1),
                                perf_mode=mybir.MatmulPerfMode.DoubleRow,
                            )
                    for s in active:
                        d2t = d2p.tile(
                            [128, JW], F16, tag="d2t", name=f"d2t{rep_i}_{it}_{s}"
                        )
                        nc.scalar.activation(
                            d2t[:],
                            psds[s][:],
                            mybir.ActivationFunctionType.Identity,
                            bias=sqb[:, it : it + 1],
                            scale=-2.0,
                        )
                        if rowacc is None:
                            rowacc = rowp.tile(
                                [128, JW], F16, tag="rowacc", name=f"ra{rep_i}_{it}"
                            )
                            nc.vector.tensor_copy(rowacc[:], d2t[:])
                        else:
                            nc.vector.tensor_tensor(
                                rowacc[:], rowacc[:], d2t[:], op=mybir.AluOpType.max
                            )
                        if s in D2S and it in _facc_its(s):
                            if s not in facc:
                                f = facp.tile(
                                    [128, JW], F16, tag="facc", name=f"facc{rep_i}_{s}"
                                )
                                nc.vector.tensor_copy(f[:], d2t[:])
                                facc[s] = f
                            else:
                                nc.vector.tensor_tensor(
                                    facc[s][:], facc[s][:], d2t[:],
                                    op=mybir.AluOpType.max,
                                )
                nc.vector.reduce_max(
                    msb[:, it : it + 1],
                    rowacc[:],
                    axis=mybir.AxisListType.X,
                    op=mybir.AluOpType.max,
                )
                if it == IT // 2 - 1:
                    emit_ar(1)
                    emit_ar(8)
            for s in sorted(D2S - {1, 8}):
                emit_ar(s)

        if HWLOOP:
            with tc.For_i(0, HWLOOP, 1):
                emit_rep(0)
                emit_rep(1)
        else:
            for rep_i in range(REPEAT):
                emit_rep(rep_i)

        nc.gpsimd.dma_start(out_m1[:], msb[:])
        nc.gpsimd.dma_start(out_m2[:], m2sb[:])

    nc.compile()
    return nc


def _get_nc():
    global _NC
    if _NC is None:
        _NC = _build_nc()
    return _NC


def _core_cols(c):
    """Global column indices (batch rows) for core c's MOVW columns."""
    segs = [np.arange(OWN) + ((c + d) % NCORES) * OWN for d in range(4)]
    p4 = ((c + 4) % NCORES) * OWN
    if c < 4:
        segs += [p4 + np.arange(512), p4 + 512 + np.arange(512)]
    else:
        segs += [p4 + 512 + np.arange(512), p4 + np.arange(512)]
    return np.concatenate(segs)


def _fold_grid(batch):
    """[128, KT, N] fp8 grid with fold channels, plus exact sq_full."""
    bT = np.ascontiguousarray(batch.T)  # [D, N] f32
    sq_full = np.einsum("ij,ij->j", bT, bT, dtype=np.float32)  # [N] exact
    b8 = bT.astype(ml_dtypes.float8_e4m3)
    g = np.ascontiguousarray(b8.reshape(KT, 128, N).transpose(1, 0, 2))
    c = (C0 - 0.5 * sq_full).astype(np.float32)
    v0 = (c / ALPHA).astype(ml_dtypes.float8_e4m3)
    v1 = (c - ALPHA * v0.astype(np.float32)).astype(ml_dtypes.float8_e4m3)
    g[126, KT - 1, :] = v0
    g[127, KT - 1, :] = v1
    return g, sq_full


def _make_in_maps(batch, positive):
    g, sq_full = _fold_grid(batch)
    in_maps = []
    for c in range(NCORES):
        cols = _core_cols(c)
        own = np.arange(OWN) + c * OWN
        mov_c = np.ascontiguousarray(g[:, :, cols]).reshape(128, KT * MOVW)
        gl = np.ascontiguousarray(g[:, :, own])
        gl[126, KT - 1, :] = np.float32(ALPHA).astype(ml_dtypes.float8_e4m3)
        gl[127, KT - 1, :] = np.float32(1.0).astype(ml_dtypes.float8_e4m3)
        movl_c = gl.reshape(128, KT * OWN)
        sqb_c = np.ascontiguousarray(
            (sq_full[own] + 2.0 * C0 + PAD).reshape(IT, 128).T
        )
        in_maps.append({"mov": mov_c, "movl": movl_c, "sqbd": sqb_c})
    return in_maps, sq_full


def _seg_lo(s, c):
    """Global start row of the batch range that m2 segment s of core c covers."""
    if s == 1:
        return c * OWN + JW
    if s < 8:
        d_, r_ = s // 2, s % 2
        return ((c + d_) % NCORES) * OWN + r_ * JW
    t = (c + 4) % NCORES
    if s == 8:
        h = 0 if c < 4 else 1
    else:
        h = 1 if c < 4 else 0
    return t * OWN + h * JW


def _combine(results, batch, positive, sq_full):
    f32 = np.float32
    d2max = np.concatenate(
        [results[c]["out_m1"].T.reshape(-1) for c in range(NCORES)]
    ).astype(f32)
    for s in range(1, 10):
        for c in range(NCORES):
            seg = results[c]["out_m2"][0, (s - 1) * JW : s * JW].astype(f32)
            lo = _seg_lo(s, c)
            d2max[lo : lo + JW] = np.maximum(d2max[lo : lo + JW], seg)

    deps2 = f32(D * EPS * EPS)
    pp = f32(np.sum(positive.astype(f32) ** 2, dtype=f32))
    dotp = batch @ positive  # [N] exact f32
    max_neg = np.sqrt(np.maximum(d2max + deps2, f32(0.0)))
    pos2 = sq_full - f32(2.0) * dotp + pp
    pos_dist = np.sqrt(np.maximum(pos2 + deps2, f32(0.0)))
    losses = np.maximum(pos_dist - max_neg + f32(MARGIN), f32(0.0))
    valid = ~np.all(batch == positive[None, :], axis=1)
    cnt = f32(valid.sum())
    total = f32(np.sum(losses[valid], dtype=f32))
    return np.asarray(total / cnt, dtype=np.float32)


def run_on_cores(batch, positive, **kwargs):
    nc = _get_nc()
    in_maps, sq_full = _make_in_maps(batch, positive)
    res = bass_utils.run_bass_kernel_spmd(
        nc, in_maps, core_ids=list(range(NCORES)), **kwargs
    )
    return res, sq_full


def kernel(batch, positive):
    batch = np.asarray(batch, dtype=np.float32)
    positive = np.asarray(positive, dtype=np.float32)
    res, sq_full = run_on_cores(batch, positive)
    return _combine(res.results, batch, positive, sq_full)


# revision 3
# speedup vs baseline: 1.0504x; 1.0504x over previous
"""Batch triplet loss on 8 TRN2 NeuronCores — v11: fold + triangle + TT-tree.

v10 -> v11:
- Own-panel triangle: slab 0 (own cols [0,512)) is computed only for
  its 0-3; the dropped (its 4-7, s0) pairs are recovered from the
  transpose side by giving slab 1 facc treatment (col-max over its 0-3),
  whose segment maps to own rows [512,1024).  -16 matmuls.
- m1 row maxes via a per-it accumulator: d2t tiles TT-max (f16 2x mode,
  ~448ns) into rowacc, one tensor_reduce per it (~615ns) instead of one
  reduce per tile.  m1cols/msb two-stage reduce eliminated.
- facc/rowacc initialized by tensor_copy of the first tile instead of
  memset + TT.
- m2 segment copies moved from ScalarE to gpsimd SBUF->SBUF DMA.
"""

import os
from contextlib import ExitStack

import ml_dtypes
import numpy as np

import concourse.bass as bass
import concourse.tile as tile
from concourse import bacc, bass_isa, bass_utils, mybir

N = 8192
D = 1024
NCORES = 8
OWN = N // NCORES       # 1024
KT = D // 128           # 8
JW = 512
NPAN = 5
MOVW = NPAN * OWN       # 5120
NSLAB = MOVW // JW      # 10
IT = OWN // 128         # 8
EPS = 1e-6
MARGIN = 0.5

ALPHA = 16.0
C0 = 512.0
PAD = 8.0

F8 = mybir.dt.float8e4
F16 = mybir.dt.float16
F32 = mybir.dt.float32

_NC = None

D2S = set(range(1, 10))          # slabs with col-max (facc) treatment
NSEG = 9


def _slab_its(s):
    if s == 0:
        return range(0, IT // 2)     # triangle: own cols A only vs rows A
    if s == 8:
        return range(0, IT // 2)
    if s == 9:
        return range(IT // 2, IT)
    return range(IT)


def _chunks(it):
    if it < IT // 2:
        return [[0, 1, 2, 3, 4], [5, 6, 7, 8]]
    return [[1, 2, 3, 4], [5, 6, 7, 9]]


def _facc_its(s):
    # which its contribute to facc[s] (col-max accumulation)
    if s == 1:
        return range(0, IT // 2)     # recovers dropped (its4-7, s0) pairs
    return _slab_its(s)


def _build_nc():
    REPEAT = int(os.environ.get("KBENCH_REPEAT", "1"))
    HWLOOP = int(os.environ.get("KBENCH_HWLOOP", "0"))  # hw-loop pair count
    nc = bacc.Bacc("TRN2", target_bir_lowering=False, debug=False)
    mov = nc.dram_tensor("mov", [128, KT * MOVW], F8, kind="ExternalInput").ap()
    movl = nc.dram_tensor("movl", [128, KT * OWN], F8, kind="ExternalInput").ap()
    sqbd = nc.dram_tensor("sqbd", [128, IT], F32, kind="ExternalInput").ap()
    out_m1 = nc.dram_tensor("out_m1", [128, IT], F32, kind="ExternalOutput").ap()
    out_m2 = nc.dram_tensor("out_m2", [1, NSEG * JW], F16, kind="ExternalOutput").ap()

    mov_v = mov.rearrange("p (k w) -> p k w", k=KT)    # [128, KT, MOVW]

    with ExitStack() as ctx:
        tc = ctx.enter_context(tile.TileContext(nc))
        big = ctx.enter_context(tc.tile_pool(name="big", bufs=1))
        d2p = ctx.enter_context(tc.tile_pool(name="d2p", bufs=10))
        rowp = ctx.enter_context(tc.tile_pool(name="rowp", bufs=3))
        facp = ctx.enter_context(tc.tile_pool(name="facp", bufs=18))
        parp = ctx.enter_context(tc.tile_pool(name="parp", bufs=4))
        resp = ctx.enter_context(tc.tile_pool(name="resp", bufs=1))
        ps_mm = ctx.enter_context(tc.tile_pool(name="ps_mm", bufs=8, space="PSUM"))

        movs2 = [
            big.tile([128, KT * MOVW], F8, tag=f"mv{h}", name=f"movs_all{h}")
            for h in range(2)
        ]
        movl2 = [
            big.tile([128, KT * OWN], F8, tag=f"ml{h}", name=f"movl{h}")
            for h in range(2)
        ]
        sqb2 = [
            resp.tile([128, IT], F32, tag=f"sqb{h}", name=f"sqb{h}") for h in range(2)
        ]
        msb = resp.tile([128, IT], F32, tag="msb", name="msb")
        m2sb = resp.tile([1, NSEG * JW], F16, tag="m2sb", name="m2sb")

        def emit_rep(rep_i):
            mv = movs2[rep_i % 2][:].rearrange("p (k w) -> p k w", k=KT)
            ml = movl2[rep_i % 2][:].rearrange("p (k w) -> p k w", k=KT)
            sqb = sqb2[rep_i % 2]
            nc.sync.dma_start(sqb[:], sqbd[:])
            nc.sync.dma_start(movl2[rep_i % 2][:], movl[:])
            for n in range(NPAN):
                nc.sync.dma_start(
                    mv[:, :, n * OWN : (n + 1) * OWN],
                    mov_v[:, :, n * OWN : (n + 1) * OWN],
                )

            facc = {}

            def emit_ar(s):
                par = parp.tile([128, JW], F16, tag="par", name=f"par{rep_i}_{s}")
                nc.gpsimd.partition_all_reduce(
                    par[:], facc[s][:], channels=128, reduce_op=bass_isa.ReduceOp.max
                )
                nc.gpsimd.dma_start(
                    m2sb[:, (s - 1) * JW : s * JW], par[0:1, :]
                )

            for it in range(IT):
                rowacc = None
                for chunk in _chunks(it):
                    active = [s for s in chunk if it in _slab_its(s)]
                    psds = {}
                    for s in active:
                        psds[s] = ps_mm.tile(
                            [128, JW], F32, tag="psd", name=f"psd{rep_i}_{it}_{s}"
                        )
                    for t in range(KT // 2):
                        for s in active:
                            nc.tensor.matmul(
                                psds[s][:],
                                ml[:, 2 * t : 2 * t + 2, it * 128 : (it + 1) * 128],
                                mv[:, 2 * t : 2 * t + 2, s * JW : (s + 1) * JW],
                                start=(t == 0),
                                stop=(t == KT // 2 - 1),
                                perf_mode=mybir.MatmulPerfMode.DoubleRowSwInterleave,
                            )
                    for s in active:
                        d2t = d2p.tile(
                            [128, JW], F16, tag="d2t", name=f"d2t{rep_i}_{it}_{s}"
                        )
                        nc.scalar.activation(
                            d2t[:],
                            psds[s][:],
                            mybir.ActivationFunctionType.Identity,
                            bias=sqb[:, it : it + 1],
                            scale=-2.0,
                        )
                        if rowacc is None:
                            rowacc = rowp.tile(
                                [128, JW], F16, tag="rowacc", name=f"ra{rep_i}_{it}"
                            )
                            nc.vector.tensor_copy(rowacc[:], d2t[:])
                        else:
                            nc.vector.tensor_tensor(
                                rowacc[:], rowacc[:], d2t[:], op=mybir.AluOpType.max
                            )
                        if s in D2S and it in _facc_its(s):
                            if s not in facc:
                                f = facp.tile(
                                    [128, JW], F16, tag="facc", name=f"facc{rep_i}_{s}"
                                )
                                nc.vector.tensor_copy(f[:], d2t[:])
                                facc[s] = f
                            else:
                                nc.vector.tensor_tensor(
                                    facc[s][:], facc[s][:], d2t[:],
                                    op=mybir.AluOpType.max,
                                )
                nc.vector.reduce_max(
                    msb[:, it : it + 1],
                    rowacc[:],
                    axis=mybir.AxisListType.X,
                    op=mybir.AluOpType.max,
                )
                if it == IT // 2 - 1:
                    emit_ar(1)
                    emit_ar(8)
            for s in sorted(D2S - {1, 8}):
                emit_ar(s)

        if HWLOOP:
            with tc.For_i(0, HWLOOP, 1):
                emit_rep(0)
                emit_rep(1)
        else:
            for rep_i in range(REPEAT):
                emit_rep(rep_i)

        nc.gpsimd.dma_start(out_m1[:], msb[:])
        nc.gpsimd.dma_start(out_m2[:], m2sb[:])

    nc.compile()
    return nc


def _get_nc():
    global _NC
    if _NC is None:
        _NC = _build_nc()
    return _NC


def _core_cols(c):
    """Global column indices (batch rows) for core c's MOVW columns."""
    segs = [np.arange(OWN) + ((c + d) % NCORES) * OWN for d in range(4)]
    p4 = ((c + 4) % NCORES) * OWN
    if c < 4:
        segs += [p4 + np.arange(512), p4 + 512 + np.arange(512)]
    else:
        segs += [p4 + 512 + np.arange(512), p4 + np.arange(512)]
    return np.concatenate(segs)


def _fold_grid(batch):
    """[128, KT, N] fp8 grid with fold channels, plus exact sq_full."""
    bT = np.ascontiguousarray(batch.T)  # [D, N] f32
    sq_full = np.einsum("ij,ij->j", bT, bT, dtype=np.float32)  # [N] exact
    b8 = bT.astype(ml_dtypes.float8_e4m3)
    g = np.ascontiguousarray(b8.reshape(KT, 128, N).transpose(1, 0, 2))
    c = (C0 - 0.5 * sq_full).astype(np.float32)
    v0 = (c / ALPHA).astype(ml_dtypes.float8_e4m3)
    v1 = (c - ALPHA * v0.astype(np.float32)).astype(ml_dtypes.float8_e4m3)
    g[126, KT - 1, :] = v0
    g[127, KT - 1, :] = v1
    return g, sq_full


def _make_in_maps(batch, positive):
    g, sq_full = _fold_grid(batch)
    in_maps = []
    for c in range(NCORES):
        cols = _core_cols(c)
        own = np.arange(OWN) + c * OWN
        mov_c = np.ascontiguousarray(g[:, :, cols]).reshape(128, KT * MOVW)
        gl = np.ascontiguousarray(g[:, :, own])
        gl[126, KT - 1, :] = np.float32(ALPHA).astype(ml_dtypes.float8_e4m3)
        gl[127, KT - 1, :] = np.float32(1.0).astype(ml_dtypes.float8_e4m3)
        # SwInterleave weight layout: per (t, it) 128x256 block, walk order
        # [pair0 cols, pair1 cols] must read A127,B127,A126,B126,...,A0,B0
        gl5 = gl.reshape(128, KT // 2, 2, IT, 128)
        A = gl5[:, :, 0, :, ::-1]
        B = gl5[:, :, 1, :, ::-1]
        inter = np.empty((128, KT // 2, IT, 256), dtype=gl.dtype)
        inter[..., 0::2] = A
        inter[..., 1::2] = B
        mlsw = np.empty((128, KT // 2, 2, IT, 128), dtype=gl.dtype)
        mlsw[:, :, 0] = inter[..., :128]
        mlsw[:, :, 1] = inter[..., 128:]
        movl_c = np.ascontiguousarray(mlsw).reshape(128, KT * OWN)
        sqb_c = np.ascontiguousarray(
            (sq_full[own] + 2.0 * C0 + PAD).reshape(IT, 128).T
        )
        in_maps.append({"mov": mov_c, "movl": movl_c, "sqbd": sqb_c})
    return in_maps, sq_full


def _seg_lo(s, c):
    """Global start row of the batch range that m2 segment s of core c covers."""
    if s == 1:
        return c * OWN + JW
    if s < 8:
        d_, r_ = s // 2, s % 2
        return ((c + d_) % NCORES) * OWN + r_ * JW
    t = (c + 4) % NCORES
    if s == 8:
        h = 0 if c < 4 else 1
    else:
        h = 1 if c < 4 else 0
    return t * OWN + h * JW


def _combine(results, batch, positive, sq_full):
    f32 = np.float32
    d2max = np.concatenate(
        [results[c]["out_m1"].T.reshape(-1) for c in range(NCORES)]
    ).astype(f32)
    for s in range(1, 10):
        for c in range(NCORES):
            seg = results[c]["out_m2"][0, (s - 1) * JW : s * JW].astype(f32)
            lo = _seg_lo(s, c)
            d2max[lo : lo + JW] = np.maximum(d2max[lo : lo + JW], seg)

    deps2 = f32(D * EPS * EPS)
    pp = f32(np.sum(positive.astype(f32) ** 2, dtype=f32))
    dotp = batch @ positive  # [N] exact f32
    max_neg = np.sqrt(np.maximum(d2max + deps2, f32(0.0)))
    pos2 = sq_full - f32(2.0) * dotp + pp
    pos_dist = np.sqrt(np.maximum(pos2 + deps2, f32(0.0)))
    losses = np.maximum(pos_dist - max_neg + f32(MARGIN), f32(0.0))
    valid = ~np.all(batch == positive[None, :], axis=1)
    cnt = f32(valid.sum())
    total = f32(np.sum(losses[valid], dtype=f32))
    return np.asarray(total / cnt, dtype=np.float32)


def run_on_cores(batch, positive, **kwargs):
    nc = _get_nc()
    in_maps, sq_full = _make_in_maps(batch, positive)
    res = bass_utils.run_bass_kernel_spmd(
        nc, in_maps, core_ids=list(range(NCORES)), **kwargs
    )
    return res, sq_full


def kernel(batch, positive):
    batch = np.asarray(batch, dtype=np.float32)
    positive = np.asarray(positive, dtype=np.float32)
    res, sq_full = run_on_cores(batch, positive)
    return _combine(res.results, batch, positive, sq_full)
